# revision 15
# baseline (speedup 1.0000x reference)
"""Per-donor routed linear layer on 8 Trainium2 cores.

out[i] = x[i] @ W[donor_labels[i]].T + b[donor_labels[i]]

Strategy: route on host (stable sort of rows by donor label), one donor per
core, zero-padded to a common row count R.  Each core runs a dense
(R x 1024) @ (1024 x 100) matmul with K tiled 8x128, accumulating in PSUM,
bias added on the DVE during PSUM->SBUF eviction.

x is pre-permuted on host into a blocked, feature-major layout
(NB, 128 partitions, 8 k-tiles, 512 rows) so each SBUF partition's DMA read
per block is one contiguous run (8KB at fp16, 16KB at fp32 — short 2KB runs
measured only ~20GB/s per SDMA engine vs ~23-27 at line rate).  The output
comes back gene-major (100, R), stored in groups of 8 blocks (16KB runs),
and is transposed/unpermuted on host.

Default mode "f16" casts x and W to fp16 on host: halves HBM traffic (the
kernel is memory-bound) and runs the PE at 1 cycle/row.  fp16's 11-bit
mantissa on N(0,1)-scale data gives rel err ~3.1e-4 vs the fp32 reference
(measured, deterministic).  Set BEC_MM_DTYPE=f32 for exact fp32 (~1.9x
slower), f32r or f16w2 for intermediate accuracy/speed points.
"""

import os
import sys

sys.path.insert(0, "/opt/trn_rl_repo")

import numpy as np

import concourse.bacc as bacc
import concourse.mybir as mybir
from concourse.tile import TileContext
from concourse.bass_utils import run_bass_kernel_spmd

N_CORES = 8
N_DONORS = 8
D_IN = 1024
N_GENES = 100
K_TILES = D_IN // 128
BLOCK = 512  # moving rows per matmul (one fp32 PSUM bank)
OG = 4  # output blocks grouped per store DMA (8KB runs per partition)

# "f32"   = exact fp32 matmul (4 cyc/row, rel err ~9e-7)
# "f32r"  = fast fp32 (~2.5 cyc/row effective, rel err ~1.6e-4 measured)
# "f16"   = fp16 x and W (1 cyc/row, half DMA traffic, rel err ~3.1e-4)
# "f16w2" = fp16 x, W split into fp16 hi+lo (2 matmuls/k-tile; W error
#           cancels, rel err ~1.5e-4 at fp16 speed)
MM_DTYPE = os.environ.get("BEC_MM_DTYPE", "f16")
NP_IN_DT = {
    "f32": np.float32,
    "f32r": np.float32,
    "f16": np.float16,
    "f16w2": np.float16,
}[MM_DTYPE]
W_TERMS = 2 if MM_DTYPE == "f16w2" else 1


def _build_program(R: int):
    nc = bacc.Bacc(
        "TRN2",
        target_bir_lowering=False,
        debug=False,
        enable_asserts=False,
        num_devices=N_CORES,
    )
    mm_dt = {
        "f32": mybir.dt.float32,
        "f32r": mybir.dt.float32r,
        "f16": mybir.dt.float16,
        "f16w2": mybir.dt.float16,
    }[MM_DTYPE]
    n_blocks = R // BLOCK

    xb = nc.dram_tensor(
        "xb", (n_blocks, 128, K_TILES, BLOCK), mm_dt, kind="ExternalInput"
    ).ap()
    wb = nc.dram_tensor(
        "wb", (W_TERMS, 128, K_TILES, N_GENES), mm_dt, kind="ExternalInput"
    ).ap()
    bias = nc.dram_tensor(
        "bias", (N_GENES, 1), mybir.dt.float32, kind="ExternalInput"
    ).ap()
    outt = nc.dram_tensor(
        "outt", (N_GENES, R), mybir.dt.float32, kind="ExternalOutput"
    ).ap()

    with TileContext(nc) as tc:
        with (
            tc.tile_pool(name="const", bufs=1) as const_pool,
            tc.tile_pool(name="xp", bufs=8) as x_pool,
            tc.tile_pool(name="op", bufs=3) as out_pool,
            tc.tile_pool(name="ps", bufs=6, space="PSUM") as psum_pool,
        ):
            w_tiles = []
            for t in range(W_TERMS):
                w_t = const_pool.tile([128, K_TILES, N_GENES], mm_dt, tag=f"w{t}")
                nc.scalar.dma_start(out=w_t[:], in_=wb[t])
                w_tiles.append(w_t)
            b_tile = const_pool.tile([N_GENES, 1], mybir.dt.float32)
            nc.scalar.dma_start(out=b_tile[:], in_=bias[:])

            for j in range(n_blocks):
                x_tile = x_pool.tile([128, K_TILES, BLOCK], mm_dt)
                nc.sync.dma_start(out=x_tile[:], in_=xb[j])
                psum = psum_pool.tile([N_GENES, BLOCK], mybir.dt.float32)
                n_mm = K_TILES * W_TERMS
                mi = 0
                for t in range(W_TERMS):
                    for k in range(K_TILES):
                        nc.tensor.matmul(
                            out=psum[:],
                            lhsT=w_tiles[t][:, k, :],
                            rhs=x_tile[:, k, :],
                            start=(mi == 0),
                            stop=(mi == n_mm - 1),
                        )
                        mi += 1
                if j % OG == 0:
                    gsize = min(OG, n_blocks - j)
                    o_tile = out_pool.tile([N_GENES, OG, BLOCK], mybir.dt.float32)
                nc.vector.tensor_scalar_add(
                    out=o_tile[:, j % OG, :], in0=psum[:], scalar1=b_tile[:]
                )
                if j % OG == gsize - 1:
                    g0 = (j // OG) * OG * BLOCK
                    nc.scalar.dma_start(
                        out=outt[:, g0 : g0 + gsize * BLOCK],
                        in_=o_tile[:, :gsize, :],
                    )

    nc.compile()
    return nc


def _split_w(wb32):
    """(128, K_TILES, N_GENES) fp32 -> (W_TERMS, ...) in NP_IN_DT."""
    if W_TERMS == 1:
        return wb32.astype(NP_IN_DT)[None]
    hi = wb32.astype(np.float16)
    lo = (wb32 - hi.astype(np.float32)).astype(np.float16)
    return np.stack([hi, lo])


def kernel(x, donor_labels, W, b):
    x = np.ascontiguousarray(x, dtype=np.float32)
    labels = np.asarray(donor_labels).astype(np.int64)
    W = np.asarray(W, dtype=np.float32)
    b = np.asarray(b, dtype=np.float32)
    B = x.shape[0]

    order = np.argsort(labels, kind="stable")
    counts = np.bincount(labels, minlength=N_DONORS)
    starts = np.zeros(N_DONORS + 1, dtype=np.int64)
    np.cumsum(counts, out=starts[1:])
    R = max(BLOCK, int(-(-counts.max() // BLOCK)) * BLOCK)
    n_blocks = R // BLOCK

    in_maps = []
    idx_per_core = []
    for d in range(N_CORES):
        idx = order[starts[d] : starts[d + 1]]
        idx_per_core.append(idx)
        xr = np.zeros((R, D_IN), dtype=NP_IN_DT)
        xr[: len(idx)] = x[idx].astype(NP_IN_DT)
        # (j*512+r, k*128+p) -> (j, p, k, r): one contiguous 16KB run per
        # partition per block on the device side.
        xb = np.ascontiguousarray(
            xr.reshape(n_blocks, BLOCK, K_TILES, 128).transpose(0, 3, 2, 1)
        )
        in_maps.append(
            {
                "xb": xb,
                "wb": _split_w(
                    np.ascontiguousarray(
                        W[d].T.reshape(K_TILES, 128, N_GENES).transpose(1, 0, 2)
                    )
                ),
                "bias": np.ascontiguousarray(b[d].reshape(N_GENES, 1)),
            }
        )

    nc = _build_program(R)
    try:
        res = run_bass_kernel_spmd(nc, in_maps, core_ids=list(range(N_CORES)))
    except Exception:
        # One retry: the axon-tunneled device occasionally drops a run with a
        # transient NRT exec error; a fresh dispatch succeeds.
        res = run_bass_kernel_spmd(nc, in_maps, core_ids=list(range(N_CORES)))

    out = np.empty((B, N_GENES), dtype=np.float32)
    for d in range(N_CORES):
        idx = idx_per_core[d]
        out[idx] = res.results[d]["outt"][:, : len(idx)].T
    return out
